# revision 12
# baseline (speedup 1.0000x reference)
"""Trainium2 Bass kernel for nn_AtteMatchLay (multi-perspective cosine matching).

Math (per flattened row n, perspective p):
    dot[n,p] = sum_d r[n,d]*m[n,d]*w2[p,d]
    n1s[n,p] = sum_d r[n,d]^2 * w2[p,d]        (w2 = weight**2)
    n2s[n,p] = sum_d m[n,d]^2 * w2[p,d]
    cos[n,p] = dot / (sqrt(n1s)*sqrt(n2s))

Strategy: data-parallel over the flattened N=16*512=8192 rows across 8 cores
(1024 rows each), contraction dim D on SBUF partitions (6 blocks of 128).

Scheduling facts this version is built around (trace + cost-model driven):
  * DVE's 2-port perf mode shares an exclusive-lock SBUF port pair with
    GpSimd; concurrent GpSimd tensor ops stall DVE ~4x. Products run on
    DVE (rm, mm) + ACT (rr squares) only; GpSimd only does tiny memsets
    at startup.
  * The ACT sequencer's exec queue depth is 0, so DMA triggers issued from
    the ACT queue serialize against ACT engine ops. All triggers go on the
    SP queue only. One trigger costs ~0.6us of SP time, so r and m are
    packed interleaved into ONE DRAM tensor; each trigger lands a matched
    (r,m) chunk.
  * Column-staggered stream: the first half of the stream carries columns
    0-511 of ALL six d-blocks, so PSUM group 0 completes mid-stream and its
    whole epilogue + output DMA run overlapped with the second half of the
    stream. Columns 512-1023 follow, with block 5 split into two quarter
    chunks so the final chain is products[128,256] -> 3 matmuls ->
    arsqrt/mults on [20,256] -> 10KB output DMA.
  * The abs_reciprocal_sqrt_and_small ACT table also serves `square`; a
    dummy ARSQRT before any Square makes the single table load happen
    during the DMA-wait window instead of a 1.3us reload on the tail.
  * PSUM groups are column-disjoint tiles (512/256/256) so epilogue reads
    never create WAR hazards against later accumulating matmuls.
"""

import sys

if "/opt/trn_rl_repo" not in sys.path:
    sys.path.insert(0, "/opt/trn_rl_repo")

import numpy as np

# ---- problem constants (hardcoded per contract) ----
BSZ, SL, D, MP = 16, 512, 768, 20
N = BSZ * SL           # 8192 flattened rows
NCORES = 8
NSH = N // NCORES      # 1024 rows per core
P = 128                # SBUF partitions
NB = D // P            # 6 d-blocks
HW_ = 512              # half width (columns)
QW = 256               # quarter width

# column groups: [0:512], [512:768], [768:1024]
GRP = [(0, 512), (512, 256), (768, 256)]

_CACHE = {}


def _build():
    import concourse.tile as tile
    from concourse import bacc, mybir

    f32 = mybir.dt.float32
    bf16 = mybir.dt.bfloat16
    nc = bacc.Bacc(None, target_bir_lowering=False)

    xD = nc.dram_tensor("xD", [P, 2 * NB * NSH], bf16, kind="ExternalInput")
    w2D = nc.dram_tensor("w2D", [P, NB * MP], bf16, kind="ExternalInput")
    out = nc.dram_tensor("out", [MP, NSH], bf16, kind="ExternalOutput")

    SQ = mybir.ActivationFunctionType.Square
    ARSQRT = mybir.ActivationFunctionType.Abs_reciprocal_sqrt
    MUL = mybir.AluOpType.mult

    with tile.TileContext(nc) as tc:
        with (
            tc.tile_pool(name="const", bufs=1) as const,
            tc.tile_pool(name="inp", bufs=1) as inp,
            tc.tile_pool(name="prod", bufs=3) as prod,
            tc.tile_pool(name="epi", bufs=1) as epi,
            tc.tile_pool(name="psum", bufs=1, space="PSUM") as psum,
        ):
            w2_sb = const.tile([P, NB, MP], bf16, tag="w2")
            # xh[h]: [128, block, r/m, 512] for column half h
            xh0 = inp.tile([P, NB, 2, HW_], bf16, tag="xh0")
            xh1 = inp.tile([P, NB, 2, HW_], bf16, tag="xh1")
            bias_b = const.tile([P, 1], bf16, tag="bias_b")
            bias_f = const.tile([MP, 1], f32, tag="bias_f")
            dum = const.tile([MP, 1], f32, tag="dum")
            nc.gpsimd.memset(bias_b[:], 0.0)
            nc.gpsimd.memset(bias_f[:], 0.0)
            nc.gpsimd.memset(dum[:], 1.0)

            # Force the abs_reciprocal_sqrt_and_small table (also serves
            # Square) to load once, now, hidden under the DMA wait.
            nc.scalar.activation(dum[:], dum[:], ARSQRT, bias=bias_f[:])

            # ---- DMA triggers (all on SP, in stream order) ----
            # chunk list: (dst tile, block range, col range within half)
            # half 0: [b0], [b1,b2], [b3,b4], [b5]
            # half 1: [b0,b1], [b2,b3], [b4], [b5 q0], [b5 q1]
            def chunk(dst, off, b0, b1, c0=0, c1=HW_):
                w = c1 - c0
                nc.sync.dma_start(
                    out=dst[:, b0:b1, :, c0:c1],
                    in_=xD[:, off : off + 2 * (b1 - b0) * w].rearrange(
                        "p (b t n) -> p b t n", b=b1 - b0, t=2
                    ),
                )
                return off + 2 * (b1 - b0) * w

            off = 0
            off = chunk(xh0, off, 0, 1)          # b0 h0 (earliest compute)
            nc.sync.dma_start(
                out=w2_sb[:], in_=w2D[:, :].rearrange("p (b q) -> p b q", b=NB)
            )
            off = chunk(xh0, off, 1, 3)          # b1,b2 h0
            off = chunk(xh0, off, 3, 5)          # b3,b4 h0
            off = chunk(xh0, off, 5, 6)          # b5 h0
            off = chunk(xh1, off, 0, 2)          # b0,b1 h1
            off = chunk(xh1, off, 2, 4)          # b2,b3 h1
            off = chunk(xh1, off, 4, 5)          # b4 h1
            off = chunk(xh1, off, 5, 6, 0, QW)   # b5 h1 q0 (cols 512-767)
            off = chunk(xh1, off, 5, 6, QW, HW_) # b5 h1 q1 (cols 768-1023)

            # ---- PSUM accumulators ----
            # PSUM tiles are bank-granular (2KB = 512 f32), so the two
            # quarter groups share bank-aligned tiles via column slices.
            dotA = psum.tile([MP, HW_], f32, tag="dotA")
            n1A = psum.tile([MP, HW_], f32, tag="n1A")
            n2A = psum.tile([MP, HW_], f32, tag="n2A")
            dotB = psum.tile([MP, HW_], f32, tag="dotB")
            n1B = psum.tile([MP, HW_], f32, tag="n1B")
            n2B = psum.tile([MP, HW_], f32, tag="n2B")
            # (dot tile, n1 tile, n2 tile, column base within tile)
            ps = [
                (dotA, n1A, n2A, 0),
                (dotB, n1B, n2B, 0),
                (dotB, n1B, n2B, QW),
            ]

            u1 = epi.tile([MP, NSH], f32, tag="u1")
            u2 = epi.tile([MP, NSH], f32, tag="u2")
            tt = epi.tile([MP, NSH], f32, tag="tt")
            cos = epi.tile([MP, NSH], bf16, tag="cos")

            def products(xt, b, c0, c1, tag):
                w = c1 - c0
                rsl = xt[:, b, 0, c0:c1]
                msl = xt[:, b, 1, c0:c1]
                rm = prod.tile([P, w], bf16, tag=f"rm{tag}")
                rr = prod.tile([P, w], bf16, tag=f"rr{tag}")
                mm = prod.tile([P, w], bf16, tag=f"mm{tag}")
                nc.vector.tensor_tensor(rm[:], rsl, msl, MUL)
                nc.scalar.activation(rr[:], rsl, SQ, bias=bias_b[:])
                nc.vector.tensor_tensor(mm[:], msl, msl, MUL)
                return rm, rr, mm

            def matmuls(rm, rr, mm, b, gi, pc0, pc1, oc0, st, sp):
                # pc0:pc1 = column slice within the product tiles;
                # oc0 = destination offset within group gi's PSUM region.
                dps, n1p, n2p, base = ps[gi]
                o0 = base + oc0
                o1 = o0 + (pc1 - pc0)
                w2b = w2_sb[:, b, :]
                kw = dict(start=st, stop=sp, skip_group_check=True)
                nc.tensor.matmul(dps[:, o0:o1], w2b, rm[:, pc0:pc1], **kw)
                nc.tensor.matmul(n1p[:, o0:o1], w2b, rr[:, pc0:pc1], **kw)
                nc.tensor.matmul(n2p[:, o0:o1], w2b, mm[:, pc0:pc1], **kw)

            def epilogue(gi, out_dma=True):
                gc, gw = GRP[gi]
                gs = slice(gc, gc + gw)
                dps, n1p, n2p, base = ps[gi]
                bs = slice(base, base + gw)
                nc.scalar.activation(u1[:, gs], n1p[:, bs], ARSQRT, bias=bias_f[:])
                nc.scalar.activation(u2[:, gs], n2p[:, bs], ARSQRT, bias=bias_f[:])
                nc.vector.tensor_tensor(tt[:, gs], u1[:, gs], u2[:, gs], MUL)
                nc.vector.tensor_tensor(cos[:, gs], dps[:, bs], tt[:, gs], MUL)
                if out_dma:
                    nc.sync.dma_start(out=out[:, gs], in_=cos[:, gs])

            # ---- half 0: full 512-wide pipeline, epilogue mid-stream ----
            for b in range(NB):
                rm, rr, mm = products(xh0, b, 0, HW_, "h0")
                matmuls(rm, rr, mm, b, 0, 0, HW_, 0, st=b == 0, sp=b == NB - 1)
            epilogue(0)

            # ---- half 1: blocks 0..4 wide, feeding groups 1 and 2 ----
            # matmul start=True resets the ENTIRE PSUM bank, so only the
            # bank's first writer (group 1 at b0) may use it; group 2
            # accumulates onto the zeroed upper half.
            for b in range(NB - 1):
                rm, rr, mm = products(xh1, b, 0, HW_, "h1")
                matmuls(rm, rr, mm, b, 1, 0, QW, 0, st=b == 0, sp=False)
                matmuls(rm, rr, mm, b, 2, QW, HW_, 0, st=False, sp=False)

            # ---- block 5 of half 1: two quarter chains ----
            for qi, gi in ((0, 1), (1, 2)):
                c0, c1 = qi * QW, (qi + 1) * QW
                rm, rr, mm = products(xh1, NB - 1, c0, c1, f"q{qi}")
                matmuls(rm, rr, mm, NB - 1, gi, 0, QW, 0, st=False, sp=True)
                epilogue(gi)

    nc.finalize()
    return nc


def get_nc():
    if "nc" not in _CACHE:
        _CACHE["nc"] = _build()
    return _CACHE["nc"]


def _pack_pair(r2d, m2d):
    # [1024 rows, 768] f32 x2 -> [128, 12288] bf16 in stream-chunk order.
    import ml_dtypes

    rt = r2d.T.reshape(NB, P, NSH)  # [b, p, n]
    mt = m2d.T.reshape(NB, P, NSH)

    parts = []

    def chunk(b0, b1, c0, c1):
        for b in range(b0, b1):
            parts.append(rt[b][:, c0:c1])
            parts.append(mt[b][:, c0:c1])

    chunk(0, 1, 0, HW_)
    chunk(1, 3, 0, HW_)
    chunk(3, 5, 0, HW_)
    chunk(5, 6, 0, HW_)
    chunk(0, 2, HW_, NSH)
    chunk(2, 4, HW_, NSH)
    chunk(4, 5, HW_, NSH)
    chunk(5, 6, HW_, HW_ + QW)
    chunk(5, 6, HW_ + QW, NSH)
    x = np.concatenate(parts, axis=1)  # [P, 2*NB*NSH]
    return np.ascontiguousarray(x.astype(ml_dtypes.bfloat16))


def make_in_maps(repres, max_att, weight):
    import ml_dtypes

    r = np.ascontiguousarray(repres, dtype=np.float32).reshape(N, D)
    m = np.ascontiguousarray(max_att, dtype=np.float32).reshape(N, D)
    w2t = (weight.astype(np.float32) ** 2).T  # [D, MP]
    w2d = np.ascontiguousarray(
        w2t.reshape(NB, P, MP).transpose(1, 0, 2).reshape(P, NB * MP)
        .astype(ml_dtypes.bfloat16)
    )
    in_maps = []
    for c in range(NCORES):
        rows = slice(c * NSH, (c + 1) * NSH)
        in_maps.append({"xD": _pack_pair(r[rows], m[rows]), "w2D": w2d})
    return in_maps


def gather(results):
    # results: list of dicts with "out" [MP, NSH] bf16 per core -> [BSZ, SL, MP] f32
    cols = np.concatenate(
        [results[c]["out"].astype(np.float32) for c in range(NCORES)], axis=1
    )
    return np.ascontiguousarray(cols.T).reshape(BSZ, SL, MP)


def kernel(repres, max_att, weight, **kw):
    from concourse.bass_utils import run_bass_kernel_spmd

    nc = get_nc()
    in_maps = make_in_maps(repres, max_att, weight)
    res = run_bass_kernel_spmd(nc, in_maps, list(range(NCORES)))
    return gather(res.results)


# revision 13
# speedup vs baseline: 1.0819x; 1.0819x over previous
"""Trainium2 Bass kernel for nn_AtteMatchLay (multi-perspective cosine matching).

Math (per flattened row n, perspective p):
    dot[n,p] = sum_d r[n,d]*m[n,d]*w2[p,d]
    n1s[n,p] = sum_d r[n,d]^2 * w2[p,d]        (w2 = weight**2)
    n2s[n,p] = sum_d m[n,d]^2 * w2[p,d]
    cos[n,p] = dot / (sqrt(n1s)*sqrt(n2s))

Strategy: data-parallel over the flattened N=16*512=8192 rows across 8 cores
(1024 rows each), contraction dim D on SBUF partitions (6 blocks of 128).

This kernel is DVE+ACT throughput-bound: the three elementwise products are
~13.5 engine-seconds over the two usable engines (GpSimd shares an
exclusive-lock SBUF port pair with DVE's 2-port mode and would stall it).
Everything is organized to (a) start those engines as early as possible,
(b) keep ops at full 1024 width for efficiency, (c) keep the post-stream
tail chain short.

  * r and m are packed interleaved into ONE DRAM tensor; each SP-queue
    trigger lands a matched (r,m) block pair (ACT queue must stay free:
    its sequencer serializes triggers against engine ops). Block 0 is the
    very first trigger so products start ~1.5us earlier; block 5 streams
    last as four quarter chunks so the final chain is short.
  * Products: rm+mm on DVE (2x bf16), rr on ACT (Square). Full width for
    b0..b4, quarter width for b5.
  * PSUM: two column groups x {dot,n1,n2} in their own banks (matmul
    start=True resets a whole bank, so groups never share banks).
  * Epilogue per group: u1,u2 = ARSQRT(n1s,n2s) on ACT (PSUM-direct,
    bf16 out), t = u1*u2 (DVE bf16 2x), cos = dot*t (bf16 out), DMA out.
    A dummy ARSQRT issued before any Square pins the one ACT table
    (abs_reciprocal_sqrt_and_small serves both) during the DMA-wait
    window instead of a 1.3us reload on the tail.
"""

import sys

if "/opt/trn_rl_repo" not in sys.path:
    sys.path.insert(0, "/opt/trn_rl_repo")

import numpy as np

# ---- problem constants (hardcoded per contract) ----
BSZ, SL, D, MP = 16, 512, 768, 20
N = BSZ * SL           # 8192 flattened rows
NCORES = 8
NSH = N // NCORES      # 1024 rows per core
P = 128                # SBUF partitions
NB = D // P            # 6 d-blocks
NBF = NB - 1           # blocks streamed full width
G = 2                  # PSUM column groups
GW = NSH // G          # 512
Q = 4                  # tail quarters of block 5
QW = NSH // Q          # 256

_CACHE = {}


def _build():
    import concourse.tile as tile
    from concourse import bacc, mybir

    f32 = mybir.dt.float32
    bf16 = mybir.dt.bfloat16
    nc = bacc.Bacc(None, target_bir_lowering=False)

    xD = nc.dram_tensor("xD", [P, 2 * NB * NSH], bf16, kind="ExternalInput")
    w2D = nc.dram_tensor("w2D", [P, NB * MP], bf16, kind="ExternalInput")
    out = nc.dram_tensor("out", [MP, NSH], bf16, kind="ExternalOutput")

    SQ = mybir.ActivationFunctionType.Square
    ARSQRT = mybir.ActivationFunctionType.Abs_reciprocal_sqrt
    MUL = mybir.AluOpType.mult

    with tile.TileContext(nc) as tc:
        with (
            tc.tile_pool(name="const", bufs=1) as const,
            tc.tile_pool(name="inp", bufs=1) as inp,
            tc.tile_pool(name="prod", bufs=3) as prod,
            tc.tile_pool(name="epi", bufs=1) as epi,
            tc.tile_pool(name="psum", bufs=1, space="PSUM") as psum,
        ):
            w2_sb = const.tile([P, NB, MP], bf16, tag="w2")
            x_sb = inp.tile([P, NB, 2, NSH], bf16, tag="x")
            bias_b = const.tile([P, 1], bf16, tag="bias_b")
            bias_f = const.tile([MP, 1], f32, tag="bias_f")
            dum = const.tile([MP, 1], f32, tag="dum")
            nc.gpsimd.memset(bias_b[:], 0.0)
            nc.gpsimd.memset(bias_f[:], 0.0)
            nc.gpsimd.memset(dum[:], 1.0)

            nc.scalar.activation(dum[:], dum[:], ARSQRT, bias=bias_f[:])

            # ---- DMA triggers (all SP queue, stream order) ----
            def ld(b, c0, c1):
                # chunk holds [r-cols | m-cols] of block b, columns c0:c1
                w = c1 - c0
                off = ld.off
                nc.sync.dma_start(
                    out=x_sb[:, b, :, c0:c1],
                    in_=xD[:, off : off + 2 * w].rearrange("p (t n) -> p t n", t=2),
                )
                ld.off = off + 2 * w

            ld.off = 0
            ld(0, 0, NSH)                       # block 0 first: earliest compute
            nc.sync.dma_start(
                out=w2_sb[:], in_=w2D[:, :].rearrange("p (b q) -> p b q", b=NB)
            )
            for b in range(1, NBF):
                ld(b, 0, NSH)
            for q in range(Q):
                ld(NB - 1, q * QW, (q + 1) * QW)

            # ---- PSUM accumulators: per-group banks ----
            dot_ps, n1_ps, n2_ps = [], [], []
            for g in range(G):
                dps = psum.tile([MP, GW], f32, tag=f"dot{g}")
                n1p = psum.tile([MP, GW], f32, tag=f"n1{g}")
                n2p = psum.tile([MP, GW], f32, tag=f"n2{g}")
                dot_ps.append(dps)
                n1_ps.append(n1p)
                n2_ps.append(n2p)

            u1 = epi.tile([MP, NSH], bf16, tag="u1")
            u2 = epi.tile([MP, NSH], bf16, tag="u2")
            tt = epi.tile([MP, NSH], bf16, tag="tt")
            cos = epi.tile([MP, NSH], bf16, tag="cos")

            # ---- blocks 0..4: full-width products + 6 matmuls each ----
            for b in range(NBF):
                rsl = x_sb[:, b, 0, :]
                msl = x_sb[:, b, 1, :]
                rm = prod.tile([P, NSH], bf16, tag="rm")
                rr = prod.tile([P, NSH], bf16, tag="rr")
                mm = prod.tile([P, NSH], bf16, tag="mm")
                nc.vector.tensor_tensor(rm[:], rsl, msl, MUL)
                nc.scalar.activation(rr[:], rsl, SQ, bias=bias_b[:])
                nc.vector.tensor_tensor(mm[:], msl, msl, MUL)
                w2b = w2_sb[:, b, :]
                st = b == 0
                for g in range(G):
                    gsl = slice(g * GW, (g + 1) * GW)
                    nc.tensor.matmul(dot_ps[g][:], w2b, rm[:, gsl], start=st, stop=False)
                    nc.tensor.matmul(n1_ps[g][:], w2b, rr[:, gsl], start=st, stop=False)
                    nc.tensor.matmul(n2_ps[g][:], w2b, mm[:, gsl], start=st, stop=False)

            # ---- block 5 quarters: products + finishing matmuls ----
            rm5 = prod.tile([P, NSH], bf16, tag="rm5")
            rr5 = prod.tile([P, NSH], bf16, tag="rr5")
            mm5 = prod.tile([P, NSH], bf16, tag="mm5")
            w2b5 = w2_sb[:, NB - 1, :]
            b5 = NB - 1
            for q in range(Q):
                qc = slice(q * QW, (q + 1) * QW)
                g = q // (Q // G)
                gq = slice((q * QW) % GW, (q * QW) % GW + QW)
                rq = x_sb[:, b5, 0, qc]
                mq = x_sb[:, b5, 1, qc]
                nc.vector.tensor_tensor(rm5[:, qc], rq, mq, MUL)
                nc.scalar.activation(rr5[:, qc], rq, SQ, bias=bias_b[:])
                nc.vector.tensor_tensor(mm5[:, qc], mq, mq, MUL)
                kw = dict(start=False, stop=q % 2 == 1, skip_group_check=True)
                nc.tensor.matmul(dot_ps[g][:, gq], w2b5, rm5[:, qc], **kw)
                nc.tensor.matmul(n1_ps[g][:, gq], w2b5, rr5[:, qc], **kw)
                nc.tensor.matmul(n2_ps[g][:, gq], w2b5, mm5[:, qc], **kw)

            # ---- epilogue per group + output DMA ----
            for g in range(G):
                gs = slice(g * GW, (g + 1) * GW)
                nc.scalar.activation(u1[:, gs], n1_ps[g][:], ARSQRT, bias=bias_f[:])
                nc.scalar.activation(u2[:, gs], n2_ps[g][:], ARSQRT, bias=bias_f[:])
                nc.vector.tensor_tensor(tt[:, gs], u1[:, gs], u2[:, gs], MUL)
                nc.vector.tensor_tensor(cos[:, gs], dot_ps[g][:], tt[:, gs], MUL)
                nc.sync.dma_start(out=out[:, gs], in_=cos[:, gs])

    nc.finalize()
    return nc


def get_nc():
    if "nc" not in _CACHE:
        _CACHE["nc"] = _build()
    return _CACHE["nc"]


def _pack_pair(r2d, m2d):
    # [1024 rows, 768] f32 x2 -> [128, 12288] bf16 in stream-chunk order:
    # [r-b0|m-b0] ... [r-b4|m-b4], then block 5 as 4 quarter chunks.
    import ml_dtypes

    rt = r2d.T.reshape(NB, P, NSH)  # [b, p, n]
    mt = m2d.T.reshape(NB, P, NSH)
    parts = []
    for b in range(NBF):
        parts.append(rt[b])
        parts.append(mt[b])
    for q in range(Q):
        qc = slice(q * QW, (q + 1) * QW)
        parts.append(rt[NB - 1][:, qc])
        parts.append(mt[NB - 1][:, qc])
    x = np.concatenate(parts, axis=1)
    return np.ascontiguousarray(x.astype(ml_dtypes.bfloat16))


def make_in_maps(repres, max_att, weight):
    import ml_dtypes

    r = np.ascontiguousarray(repres, dtype=np.float32).reshape(N, D)
    m = np.ascontiguousarray(max_att, dtype=np.float32).reshape(N, D)
    w2t = (weight.astype(np.float32) ** 2).T  # [D, MP]
    w2d = np.ascontiguousarray(
        w2t.reshape(NB, P, MP).transpose(1, 0, 2).reshape(P, NB * MP)
        .astype(ml_dtypes.bfloat16)
    )
    in_maps = []
    for c in range(NCORES):
        rows = slice(c * NSH, (c + 1) * NSH)
        in_maps.append({"xD": _pack_pair(r[rows], m[rows]), "w2D": w2d})
    return in_maps


def gather(results):
    cols = np.concatenate(
        [results[c]["out"].astype(np.float32) for c in range(NCORES)], axis=1
    )
    return np.ascontiguousarray(cols.T).reshape(BSZ, SL, MP)


def kernel(repres, max_att, weight, **kw):
    from concourse.bass_utils import run_bass_kernel_spmd

    nc = get_nc()
    in_maps = make_in_maps(repres, max_att, weight)
    res = run_bass_kernel_spmd(nc, in_maps, list(range(NCORES)))
    return gather(res.results)
